# revision 29
# baseline (speedup 1.0000x reference)
"""Trainium2 Bass kernel for BERT post-training baseline loss
(two CRF tagging heads + sentiment head over [64, 512, 1024] hidden states).

Strategy: data-parallel over batch across 8 NeuronCores (8 sequences/core).
Per core everything is local (no collectives). Work is pipelined in 16
half-batch units (256 tokens = 1 MiB) to minimize pipeline fill/drain:
  - hidden shard streams in natural [token, h] layout (tokens on partitions)
  - pooling: all batches accumulate into persistent [8,512] PSUM tiles via
    per-batch selector columns as the stationary operand (PE, X streaming)
  - each [128,128] block is PE-transposed (f32r) into PSUM (4 packed per
    bank), evacuated to SBUF by ScalarE/VectorE copies, then
    W[128h,6].T @ X^T[128h,256tok] accumulates logits^T per batch on the PE
  - CRF emission scores via tensor-mul + reduce on VectorE against a
    host-built masked one-hot [6,512]
Host epilogue: bias adds, sentiment head (25K flops), label-only transition
scores and mask counts, final scalar loss. All hidden-dependent math is
on-device.
"""
import sys

sys.path.insert(0, "/opt/trn_rl_repo")

import numpy as np

import concourse.bacc as bacc
import concourse.tile as tile
from concourse import mybir
from concourse import bass_utils

F32 = mybir.dt.float32
F32R = mybir.dt.float32r

B, S, H, T = 64, 512, 1024, 3
NCORES = 8
BPC = B // NCORES           # batches per core = 8
TOK = BPC * S               # tokens per core = 4096
NHC = H // 128              # h-chunks = 8
NU = 2 * BPC                # half-batch units per core = 16


def _build_nc():
    nc = bacc.Bacc("TRN2", target_bir_lowering=False, debug=False)

    x_d = nc.dram_tensor("x", [TOK, H], F32R, kind="ExternalInput")
    wc_d = nc.dram_tensor("wc", [128, NHC * 6], F32R, kind="ExternalInput")
    oh_d = nc.dram_tensor("oh", [6, TOK], F32, kind="ExternalInput")
    id_d = nc.dram_tensor("ident", [128, 128], F32R, kind="ExternalInput")
    sel_d = nc.dram_tensor("sel", [128, BPC * BPC], F32R, kind="ExternalInput")

    lg_d = nc.dram_tensor("logits_t", [6, TOK], F32, kind="ExternalOutput")
    pl_d = nc.dram_tensor("pooled", [BPC, H], F32, kind="ExternalOutput")
    em_d = nc.dram_tensor("emit", [6, BPC], F32, kind="ExternalOutput")

    # unit u = 256 tokens: [u, p, j, h], token tile j in {0,1}
    x_v = x_d.ap().rearrange("(u j p) h -> u p j h", u=NU, j=2, p=128)

    with tile.TileContext(nc) as tc:
        with tc.tile_pool(name="const", bufs=1) as cp, \
             tc.tile_pool(name="xin", bufs=6) as xp, \
             tc.tile_pool(name="xt", bufs=12) as xtp, \
             tc.tile_pool(name="small", bufs=4) as smp, \
             tc.tile_pool(name="xtps", bufs=4, space="PSUM") as xtps, \
             tc.tile_pool(name="poolps", bufs=1, space="PSUM") as plps, \
             tc.tile_pool(name="logps", bufs=2, space="PSUM") as lgps:

            wc_sb = cp.tile([128, NHC * 6], F32R, tag="wc")
            oh_sb = cp.tile([6, TOK], F32, tag="oh")
            id_sb = cp.tile([128, 128], F32R, tag="id")
            sel_sb = cp.tile([128, BPC * BPC], F32R, tag="sel")
            pl_sb = cp.tile([BPC, H], F32, tag="pooled")
            em_sb = cp.tile([6, BPC], F32, tag="emit")
            # tiny constants first (PE's queue head waits on sel/ident),
            # then the first unit's 1 MiB load; oh is only needed ~25us in
            nc.sync.dma_start(id_sb[:], id_d.ap())
            nc.sync.dma_start(sel_sb[:], sel_d.ap())
            nc.sync.dma_start(wc_sb[:], wc_d.ap())

            xb0 = xp.tile([128, 2 * H], F32R, tag="xb", name="xb0")
            xb0v = xb0[:].rearrange("p (j h) -> p j h", j=2)
            nc.sync.dma_start(xb0v[:, 0], x_v[0][:, 0])
            nc.sync.dma_start(xb0v[:, 1], x_v[0][:, 1])
            nc.sync.dma_start(oh_sb[:], oh_d.ap())

            pps = [plps.tile([BPC, 512], F32, name=f"pl{hh}", tag=f"pl{hh}")
                   for hh in range(2)]

            # HAM warm-up: open the PE clock gate (4/8 -> 8/8) before the
            # real stream and keep it open until data arrives. First burst
            # runs on constants (~3.5us in), a second keep-alive burst gates
            # on the first data piece. Garbage lands in pps[0], cleared by
            # the real pooling chain's start=True.
            for w in range(40):
                nc.tensor.matmul(
                    pps[0][:, 0:128],
                    sel_sb[:, 0:BPC],
                    id_sb[:],
                    start=True,
                    stop=True,
                )
            for w in range(6):
                nc.tensor.matmul(
                    pps[0][:, 0:128],
                    sel_sb[:, 0:BPC],
                    xb0[:, 0:128],
                    start=True,
                    stop=True,
                )

            lps = None
            for u in range(NU):
                b, half = u // 2, u % 2
                if u == 0:
                    xb = xb0
                else:
                    xb = xp.tile([128, 2 * H], F32R, tag="xb")
                    nc.sync.dma_start(
                        xb[:].rearrange("p (j h) -> p j h", j=2), x_v[u]
                    )

                # pooling: accumulate token sums into row b of the
                # persistent [8,512] psum (one per h-half)
                for hh in range(2):
                    for i in range(2):
                        nc.tensor.matmul(
                            pps[hh][:],
                            sel_sb[:, b * BPC : (b + 1) * BPC],
                            xb[:, i * H + hh * 512 : i * H + hh * 512 + 512],
                            start=(u == 0 and i == 0),
                            stop=(u == NU - 1 and i == 1),
                        )

                # transpose 16 [128,128] blocks; pack 4 (2 h-chunks x 2 token
                # tiles) per [128,512] psum bank; evacuate into the shared
                # per-(batch, h-pair) [128,1024] X^T tile (hc-major halves)
                if half == 0:
                    xts = [xtp.tile([128, 1024], F32R, tag="xt",
                                    name=f"xt{u}_{hp}")
                           for hp in range(NHC // 2)]
                for hp in range(NHC // 2):      # h-chunk pairs
                    tps = xtps.tile([128, 512], F32R, tag="xtps")
                    for k in range(2):
                        hc = 2 * hp + k
                        for i in range(2):
                            nc.tensor.transpose(
                                tps[:, k * 256 + i * 128 : k * 256 + (i + 1) * 128],
                                xb[:, i * H + hc * 128 : i * H + (hc + 1) * 128],
                                id_sb[:],
                            )
                    # dst: cols [k*512 + half*256, +256) for k in {0,1}
                    dst = xts[hp][:].rearrange(
                        "p (k t) -> p k t", k=2
                    )[:, :, half * 256 : half * 256 + 256]
                    if hp % 2 == 0:
                        nc.scalar.copy(dst, tps[:].rearrange("p (k t) -> p k t", k=2))
                    else:
                        nc.vector.tensor_copy(dst, tps[:].rearrange("p (k t) -> p k t", k=2))

                if half == 1:
                    # logits^T [6, 512] for this batch, N=512 per h-chunk
                    lps = lgps.tile([6, 512], F32, tag="lg")
                    for hc in range(NHC):
                        nc.tensor.matmul(
                            lps[:],
                            wc_sb[:, hc * 6 : (hc + 1) * 6],
                            xts[hc // 2][:, (hc % 2) * 512 : (hc % 2) * 512 + 512],
                            start=(hc == 0),
                            stop=(hc == NHC - 1),
                        )
                    lsb = smp.tile([6, 512], F32, tag="lsb")
                    nc.scalar.copy(lsb[:], lps[:])
                    nc.scalar.dma_start(
                        lg_d.ap()[:, b * S : (b + 1) * S], lsb[:]
                    )
                    # CRF emission partial sums: per-class, this batch
                    scr = smp.tile([6, 512], F32, tag="scr")
                    nc.vector.tensor_mul(
                        scr[:], lps[:], oh_sb[:, b * S : (b + 1) * S]
                    )
                    nc.vector.tensor_reduce(
                        em_sb[:, b : b + 1],
                        scr[:],
                        axis=mybir.AxisListType.X,
                        op=mybir.AluOpType.add,
                    )

            for hh in range(2):
                nc.scalar.copy(
                    pl_sb[:, hh * 512 : hh * 512 + 512], pps[hh][:]
                )
            nc.scalar.dma_start(pl_d.ap(), pl_sb[:])
            nc.scalar.dma_start(em_d.ap(), em_sb[:])

    nc.compile()
    return nc


_NC_CACHE = None


def _get_nc():
    global _NC_CACHE
    if _NC_CACHE is None:
        _NC_CACHE = _build_nc()
    return _NC_CACHE


def _prep_inputs(inputs):
    hs = np.ascontiguousarray(inputs["hidden_states"], dtype=np.float32)
    mask = inputs["attention_mask"]
    al = inputs["aspect_labels"]
    ol = inputs["opinion_labels"]
    maskf = mask.astype(np.float32)

    # combined head weights -> [128, 8*6]: column block hc holds W[hc*128:(hc+1)*128, :]
    w6 = np.concatenate(
        [np.asarray(inputs["W_aspect"], np.float32),
         np.asarray(inputs["W_opinion"], np.float32)], axis=1
    )  # [1024, 6]
    wc = np.ascontiguousarray(
        w6.reshape(NHC, 128, 6).transpose(1, 0, 2).reshape(128, NHC * 6)
    )

    # masked one-hots [B, S, 6] -> per-core [6, TOK]
    oh = np.zeros((B, S, 6), dtype=np.float32)
    bi = np.arange(B)[:, None]
    si = np.arange(S)[None, :]
    oh[bi, si, al] = maskf
    oh[bi, si, np.asarray(ol) + 3] = maskf

    ident = np.eye(128, dtype=np.float32)
    sel = np.zeros((128, BPC * BPC), dtype=np.float32)
    for b in range(BPC):
        sel[:, b * BPC + b] = 1.0

    in_maps = []
    for c in range(NCORES):
        x_c = hs[c * BPC : (c + 1) * BPC].reshape(TOK, H)
        oh_c = np.ascontiguousarray(
            oh[c * BPC : (c + 1) * BPC].reshape(TOK, 6).T
        )
        in_maps.append({"x": x_c, "wc": wc, "oh": oh_c, "ident": ident,
                       "sel": sel})
    return in_maps


def _host_epilogue(inputs, results):
    mask = np.asarray(inputs["attention_mask"])
    al = np.asarray(inputs["aspect_labels"])
    ol = np.asarray(inputs["opinion_labels"])
    b_a = np.asarray(inputs["b_aspect"], np.float32)
    b_o = np.asarray(inputs["b_opinion"], np.float32)
    W_s = np.asarray(inputs["W_sent"], np.float32)
    b_s = np.asarray(inputs["b_sent"], np.float32)
    tr_a = np.asarray(inputs["trans_aspect"], np.float32)
    tr_o = np.asarray(inputs["trans_opinion"], np.float32)
    maskf = mask.astype(np.float32)

    # logits: [6, TOK] per core -> [B, S, 6]
    lt = np.stack([r["logits_t"] for r in results])          # [8, 6, 4096]
    lg = lt.reshape(NCORES, 6, BPC, S).transpose(0, 2, 3, 1).reshape(B, S, 6)
    aspect_logits = lg[..., 0:3] + b_a
    opinion_logits = lg[..., 3:6] + b_o

    # sentiment head from pooled sums
    pooled = np.concatenate([r["pooled"] for r in results], axis=0)  # [64, 1024]
    sentiment_logits = (pooled / np.float32(S)) @ W_s + b_s

    # CRF log-likelihoods
    emit = np.stack([r["emit"] for r in results])             # [8, 6, BPC]
    emit_a = float(emit[:, 0:3, :].sum())
    emit_o = float(emit[:, 3:6, :].sum())
    # bias contribution to emission scores (zero biases in practice)
    cnt_a = np.array([(maskf * (al == c)).sum() for c in range(T)])
    cnt_o = np.array([(maskf * (ol == c)).sum() for c in range(T)])
    emit_a += float(b_a @ cnt_a)
    emit_o += float(b_o @ cnt_o)

    pm = maskf[:, 1:] * maskf[:, :-1]
    trans_a = float((tr_a[al[:, :-1], al[:, 1:]] * pm).sum())
    trans_o = float((tr_o[ol[:, :-1], ol[:, 1:]] * pm).sum())

    cnt = float(maskf.sum())
    loss = -(emit_a + trans_a) / cnt - (emit_o + trans_o) / cnt

    return (
        aspect_logits.astype(np.float32),
        opinion_logits.astype(np.float32),
        sentiment_logits.astype(np.float32),
        np.float32(loss),
    )


def _run(inputs, trace=False):
    nc = _get_nc()
    in_maps = _prep_inputs(inputs)
    res = bass_utils.run_bass_kernel_spmd(
        nc, in_maps, core_ids=list(range(NCORES)), trace=trace
    )
    return _host_epilogue(inputs, res.results), res


def kernel(**inputs):
    out, _ = _run(inputs)
    return out


# revision 30
# speedup vs baseline: 1.0089x; 1.0089x over previous
"""Trainium2 Bass kernel for BERT post-training baseline loss
(two CRF tagging heads + sentiment head over [64, 512, 1024] hidden states).

Strategy: data-parallel over batch across 8 NeuronCores (8 sequences/core).
Per core everything is local (no collectives). Work is pipelined in 16
half-batch units (256 tokens = 1 MiB) to minimize pipeline fill/drain:
  - hidden shard streams in natural [token, h] layout (tokens on partitions)
  - pooling: all batches accumulate into persistent [8,512] PSUM tiles via
    per-batch selector columns as the stationary operand (PE, X streaming)
  - each [128,128] block is PE-transposed (f32r) into PSUM (4 packed per
    bank), evacuated to SBUF by ScalarE/VectorE copies, then
    W[128h,6].T @ X^T[128h,256tok] accumulates logits^T per batch on the PE
  - CRF emission scores via tensor-mul + reduce on VectorE against a
    host-built masked one-hot [6,512]
Host epilogue: bias adds, sentiment head (25K flops), label-only transition
scores and mask counts, final scalar loss. All hidden-dependent math is
on-device.
"""
import sys

sys.path.insert(0, "/opt/trn_rl_repo")

import numpy as np

import concourse.bacc as bacc
import concourse.tile as tile
from concourse import mybir
from concourse import bass_utils

F32 = mybir.dt.float32
F32R = mybir.dt.float32r

B, S, H, T = 64, 512, 1024, 3
NCORES = 8
BPC = B // NCORES           # batches per core = 8
TOK = BPC * S               # tokens per core = 4096
NHC = H // 128              # h-chunks = 8
NU = 2 * BPC                # half-batch units per core = 16


def _build_nc():
    nc = bacc.Bacc("TRN2", target_bir_lowering=False, debug=False)

    x_d = nc.dram_tensor("x", [TOK, H], F32R, kind="ExternalInput")
    wc_d = nc.dram_tensor("wc", [128, NHC * 6], F32R, kind="ExternalInput")
    oh_d = nc.dram_tensor("oh", [6, TOK], F32, kind="ExternalInput")
    id_d = nc.dram_tensor("ident", [128, 128], F32R, kind="ExternalInput")
    sel_d = nc.dram_tensor("sel", [128, BPC * BPC], F32R, kind="ExternalInput")

    lg_d = nc.dram_tensor("logits_t", [6, TOK], F32, kind="ExternalOutput")
    pl_d = nc.dram_tensor("pooled", [BPC, H], F32, kind="ExternalOutput")
    em_d = nc.dram_tensor("emit", [6, BPC], F32, kind="ExternalOutput")

    # unit u = 256 tokens: [u, p, j, h], token tile j in {0,1}
    x_v = x_d.ap().rearrange("(u j p) h -> u p j h", u=NU, j=2, p=128)

    with tile.TileContext(nc) as tc:
        with tc.tile_pool(name="const", bufs=1) as cp, \
             tc.tile_pool(name="xin", bufs=6) as xp, \
             tc.tile_pool(name="xt", bufs=12) as xtp, \
             tc.tile_pool(name="small", bufs=4) as smp, \
             tc.tile_pool(name="xtps", bufs=4, space="PSUM") as xtps, \
             tc.tile_pool(name="poolps", bufs=1, space="PSUM") as plps, \
             tc.tile_pool(name="logps", bufs=2, space="PSUM") as lgps:

            wc_sb = cp.tile([128, NHC * 6], F32R, tag="wc")
            oh_sb = cp.tile([6, TOK], F32, tag="oh")
            id_sb = cp.tile([128, 128], F32R, tag="id")
            sel_sb = cp.tile([128, BPC * BPC], F32R, tag="sel")
            pl_sb = cp.tile([BPC, H], F32, tag="pooled")
            em_sb = cp.tile([6, BPC], F32, tag="emit")
            # tiny constants first (PE's queue head waits on sel/ident),
            # then the first unit's 1 MiB load; oh is only needed ~25us in
            nc.sync.dma_start(id_sb[:], id_d.ap())
            nc.sync.dma_start(sel_sb[:], sel_d.ap())
            nc.sync.dma_start(wc_sb[:], wc_d.ap())

            xb0 = xp.tile([128, 2 * H], F32R, tag="xb", name="xb0")
            xb0v = xb0[:].rearrange("p (j h) -> p j h", j=2)
            nc.sync.dma_start(xb0v[:, 0], x_v[0][:, 0])
            nc.sync.dma_start(xb0v[:, 1], x_v[0][:, 1])
            nc.sync.dma_start(oh_sb[:], oh_d.ap())

            pps = [plps.tile([BPC, 512], F32, name=f"pl{hh}", tag=f"pl{hh}")
                   for hh in range(2)]

            # HAM warm-up: open the PE clock gate (4/8 -> 8/8) before the
            # real stream and keep it open until data arrives. First burst
            # runs on constants (~3.5us in), a second keep-alive burst gates
            # on the first data piece. Garbage lands in pps[0], cleared by
            # the real pooling chain's start=True.
            for w in range(52):
                nc.tensor.matmul(
                    pps[0][:, 0:128],
                    sel_sb[:, 0:BPC],
                    id_sb[:],
                    start=True,
                    stop=True,
                )

            lps = None
            for u in range(NU):
                b, half = u // 2, u % 2
                if u == 0:
                    xb = xb0
                else:
                    xb = xp.tile([128, 2 * H], F32R, tag="xb")
                    nc.sync.dma_start(
                        xb[:].rearrange("p (j h) -> p j h", j=2), x_v[u]
                    )

                # pooling: accumulate token sums into row b of the
                # persistent [8,512] psum (one per h-half)
                for hh in range(2):
                    for i in range(2):
                        nc.tensor.matmul(
                            pps[hh][:],
                            sel_sb[:, b * BPC : (b + 1) * BPC],
                            xb[:, i * H + hh * 512 : i * H + hh * 512 + 512],
                            start=(u == 0 and i == 0),
                            stop=(u == NU - 1 and i == 1),
                        )

                # transpose 16 [128,128] blocks; pack 4 (2 h-chunks x 2 token
                # tiles) per [128,512] psum bank; evacuate into the shared
                # per-(batch, h-pair) [128,1024] X^T tile (hc-major halves)
                if half == 0:
                    xts = [xtp.tile([128, 1024], F32R, tag="xt",
                                    name=f"xt{u}_{hp}")
                           for hp in range(NHC // 2)]
                for hp in range(NHC // 2):      # h-chunk pairs
                    tps = xtps.tile([128, 512], F32R, tag="xtps")
                    for k in range(2):
                        hc = 2 * hp + k
                        for i in range(2):
                            nc.tensor.transpose(
                                tps[:, k * 256 + i * 128 : k * 256 + (i + 1) * 128],
                                xb[:, i * H + hc * 128 : i * H + (hc + 1) * 128],
                                id_sb[:],
                            )
                    # dst: cols [k*512 + half*256, +256) for k in {0,1}
                    dst = xts[hp][:].rearrange(
                        "p (k t) -> p k t", k=2
                    )[:, :, half * 256 : half * 256 + 256]
                    if hp % 2 == 0:
                        nc.scalar.copy(dst, tps[:].rearrange("p (k t) -> p k t", k=2))
                    else:
                        nc.vector.tensor_copy(dst, tps[:].rearrange("p (k t) -> p k t", k=2))

                if half == 1:
                    # logits^T [6, 512] for this batch, N=512 per h-chunk
                    lps = lgps.tile([6, 512], F32, tag="lg")
                    for hc in range(NHC):
                        nc.tensor.matmul(
                            lps[:],
                            wc_sb[:, hc * 6 : (hc + 1) * 6],
                            xts[hc // 2][:, (hc % 2) * 512 : (hc % 2) * 512 + 512],
                            start=(hc == 0),
                            stop=(hc == NHC - 1),
                        )
                    lsb = smp.tile([6, 512], F32, tag="lsb")
                    nc.scalar.copy(lsb[:], lps[:])
                    nc.scalar.dma_start(
                        lg_d.ap()[:, b * S : (b + 1) * S], lsb[:]
                    )
                    # CRF emission partial sums: per-class, this batch
                    scr = smp.tile([6, 512], F32, tag="scr")
                    nc.vector.tensor_mul(
                        scr[:], lps[:], oh_sb[:, b * S : (b + 1) * S]
                    )
                    nc.vector.tensor_reduce(
                        em_sb[:, b : b + 1],
                        scr[:],
                        axis=mybir.AxisListType.X,
                        op=mybir.AluOpType.add,
                    )

            for hh in range(2):
                nc.scalar.copy(
                    pl_sb[:, hh * 512 : hh * 512 + 512], pps[hh][:]
                )
            nc.scalar.dma_start(pl_d.ap(), pl_sb[:])
            nc.scalar.dma_start(em_d.ap(), em_sb[:])

    nc.compile()
    return nc


_NC_CACHE = None


def _get_nc():
    global _NC_CACHE
    if _NC_CACHE is None:
        _NC_CACHE = _build_nc()
    return _NC_CACHE


def _prep_inputs(inputs):
    hs = np.ascontiguousarray(inputs["hidden_states"], dtype=np.float32)
    mask = inputs["attention_mask"]
    al = inputs["aspect_labels"]
    ol = inputs["opinion_labels"]
    maskf = mask.astype(np.float32)

    # combined head weights -> [128, 8*6]: column block hc holds W[hc*128:(hc+1)*128, :]
    w6 = np.concatenate(
        [np.asarray(inputs["W_aspect"], np.float32),
         np.asarray(inputs["W_opinion"], np.float32)], axis=1
    )  # [1024, 6]
    wc = np.ascontiguousarray(
        w6.reshape(NHC, 128, 6).transpose(1, 0, 2).reshape(128, NHC * 6)
    )

    # masked one-hots [B, S, 6] -> per-core [6, TOK]
    oh = np.zeros((B, S, 6), dtype=np.float32)
    bi = np.arange(B)[:, None]
    si = np.arange(S)[None, :]
    oh[bi, si, al] = maskf
    oh[bi, si, np.asarray(ol) + 3] = maskf

    ident = np.eye(128, dtype=np.float32)
    sel = np.zeros((128, BPC * BPC), dtype=np.float32)
    for b in range(BPC):
        sel[:, b * BPC + b] = 1.0

    in_maps = []
    for c in range(NCORES):
        x_c = hs[c * BPC : (c + 1) * BPC].reshape(TOK, H)
        oh_c = np.ascontiguousarray(
            oh[c * BPC : (c + 1) * BPC].reshape(TOK, 6).T
        )
        in_maps.append({"x": x_c, "wc": wc, "oh": oh_c, "ident": ident,
                       "sel": sel})
    return in_maps


def _host_epilogue(inputs, results):
    mask = np.asarray(inputs["attention_mask"])
    al = np.asarray(inputs["aspect_labels"])
    ol = np.asarray(inputs["opinion_labels"])
    b_a = np.asarray(inputs["b_aspect"], np.float32)
    b_o = np.asarray(inputs["b_opinion"], np.float32)
    W_s = np.asarray(inputs["W_sent"], np.float32)
    b_s = np.asarray(inputs["b_sent"], np.float32)
    tr_a = np.asarray(inputs["trans_aspect"], np.float32)
    tr_o = np.asarray(inputs["trans_opinion"], np.float32)
    maskf = mask.astype(np.float32)

    # logits: [6, TOK] per core -> [B, S, 6]
    lt = np.stack([r["logits_t"] for r in results])          # [8, 6, 4096]
    lg = lt.reshape(NCORES, 6, BPC, S).transpose(0, 2, 3, 1).reshape(B, S, 6)
    aspect_logits = lg[..., 0:3] + b_a
    opinion_logits = lg[..., 3:6] + b_o

    # sentiment head from pooled sums
    pooled = np.concatenate([r["pooled"] for r in results], axis=0)  # [64, 1024]
    sentiment_logits = (pooled / np.float32(S)) @ W_s + b_s

    # CRF log-likelihoods
    emit = np.stack([r["emit"] for r in results])             # [8, 6, BPC]
    emit_a = float(emit[:, 0:3, :].sum())
    emit_o = float(emit[:, 3:6, :].sum())
    # bias contribution to emission scores (zero biases in practice)
    cnt_a = np.array([(maskf * (al == c)).sum() for c in range(T)])
    cnt_o = np.array([(maskf * (ol == c)).sum() for c in range(T)])
    emit_a += float(b_a @ cnt_a)
    emit_o += float(b_o @ cnt_o)

    pm = maskf[:, 1:] * maskf[:, :-1]
    trans_a = float((tr_a[al[:, :-1], al[:, 1:]] * pm).sum())
    trans_o = float((tr_o[ol[:, :-1], ol[:, 1:]] * pm).sum())

    cnt = float(maskf.sum())
    loss = -(emit_a + trans_a) / cnt - (emit_o + trans_o) / cnt

    return (
        aspect_logits.astype(np.float32),
        opinion_logits.astype(np.float32),
        sentiment_logits.astype(np.float32),
        np.float32(loss),
    )


def _run(inputs, trace=False):
    nc = _get_nc()
    in_maps = _prep_inputs(inputs)
    res = bass_utils.run_bass_kernel_spmd(
        nc, in_maps, core_ids=list(range(NCORES)), trace=trace
    )
    return _host_epilogue(inputs, res.results), res


def kernel(**inputs):
    out, _ = _run(inputs)
    return out
